# revision 33
# baseline (speedup 1.0000x reference)
"""Trainium2 Bass kernel for nn_Decoder: 2-layer GRU decoder, batch-parallel over 8 cores.

v4 design (on top of v3's transposed-gate fp8 DoubleRow formulation):
  - Shard batch 128 -> 16 rows/core, replicate weights (SBUF-resident).
  - Weight matmuls batch-major (weights moving, N=512) in fp8e4 DoubleRow;
    weights pre-scaled by SCALE=64, single scale=1/SCALE fixup inside
    sigmoid/tanh.
  - NO per-step bias/constant matmuls: the step-invariant terms
    (glob@Wg + biases, pre-scaled) are folded into the PSUM->SBUF stage,
    which becomes tensor_add instead of tensor_copy (same cost), split
    across DVE and Pool.
  - x is packed into the L0 DoubleRow chain as k-tile slot 0 of the
    augmented stationary s_xh [128, 10, B] (slot1 = zeros, slots 2-9 = h0),
    with the interleaved weight tensor wcat = [wx, 0, whh0_0..7]; the
    n-gate's input-side psum is a single (x,0) DR matmul.
  - Gate math in TRANSPOSED space (gates on 128 partitions) via PE
    transposes, elementwise update on [128, 64] tiles spread over
    DVE/Pool/ACT; hidden state produced directly in the stationary layout.
  - fp32 transposed master states; fp8 shadows for gate matmuls; fp16
    shadow of h1 for the fc head.
  - Init DMAs spread across all engine queues; tail double-buffered.
"""
import sys

sys.path.insert(0, "/opt/trn_rl_repo")
import numpy as np

import concourse.bass as bass
import concourse.mybir as mybir
import concourse.tile as tile
from concourse import bacc
from concourse.bass import ds, ts
from concourse.bass_utils import run_bass_kernel_spmd

F8 = mybir.dt.float8e4
F16 = mybir.dt.float16
F32 = mybir.dt.float32
AF = mybir.ActivationFunctionType
DR = mybir.MatmulPerfMode.DoubleRow

BS, H, D, SEQ = 128, 1024, 128, 256
NCORES = 8
B = BS // NCORES          # 16 rows per core
KH = H // 128             # 8 k-tiles over hidden dim
G3 = 3 * H                # 3072 gate cols
EMBED_DIM = 265216
TS_OFF = 3 * H
TS_LEN = SEQ * (H // 2)   # 131072
SCALE = 64.0              # fp8 weight pre-scale
INV = 1.0 / SCALE
NXT = 10                  # slots in augmented L0 stationary: x, 0, h0 k0..k7


def build_nc(n_steps=SEQ, unroll=51, static=False):
    nc = bacc.Bacc()

    d_embed = nc.declare_dram_parameter("embed", [B, EMBED_DIM], F32, isOutput=False)
    d_x0 = nc.declare_dram_parameter("x0", [B, D], F32, isOutput=False)
    d_wcat0 = nc.declare_dram_parameter("wcat0", [128, NXT, G3], F8, isOutput=False)
    d_wg0 = nc.declare_dram_parameter("wg0", [128, KH, G3], F8, isOutput=False)
    d_wih1 = nc.declare_dram_parameter("wih1", [128, KH, G3], F8, isOutput=False)
    d_whh1 = nc.declare_dram_parameter("whh1", [128, KH, G3], F8, isOutput=False)
    d_fct = nc.declare_dram_parameter("fct", [128, KH, D], F16, isOutput=False)
    d_pred = nc.declare_dram_parameter("predt", [128, 4, D], F16, isOutput=False)
    d_s0b = nc.declare_dram_parameter("s0b", [G3], F16, isOutput=False)
    d_s1b = nc.declare_dram_parameter("s1b", [G3], F16, isOutput=False)
    d_bhh0n = nc.declare_dram_parameter("bhh0n", [H], F16, isOutput=False)
    d_bhh1n = nc.declare_dram_parameter("bhh1n", [H], F16, isOutput=False)
    d_fcb = nc.declare_dram_parameter("fcb", [D, 1], F32, isOutput=False)
    d_pb2 = nc.declare_dram_parameter("pb2", [D], F32, isOutput=False)
    d_id16 = nc.declare_dram_parameter("id16", [B, B], F16, isOutput=False)
    d_id128 = nc.declare_dram_parameter("id128", [128, 128], F16, isOutput=False)
    d_out = nc.declare_dram_parameter("out", [B, SEQ, D], F32, isOutput=True)

    # DRAM scratch for init-time transposes
    d_bh0 = nc.dram_tensor("bh0", [B, H], F16)
    d_bh1 = nc.dram_tensor("bh1", [B, H], F16)
    d_bx = nc.dram_tensor("bx", [B, D], F16)
    d_bg = nc.dram_tensor("bg", [B, H], F16)

    def bcast(ap_1d, parts, n):
        return bass.AP(tensor=ap_1d.tensor, offset=ap_1d.offset,
                       ap=[[0, parts]] + list(ap_1d.ap))

    with tile.TileContext(nc) as tc:
        with (
            tc.tile_pool(name="persist", bufs=1) as pp,
            tc.tile_pool(name="tmp", bufs=2) as tp,
            tc.tile_pool(name="psum", bufs=8, space="PSUM") as qq,
        ):
            # ---------------- resident tiles ----------------
            s_wcat = pp.tile([128, NXT, G3], F8)
            s_wih1 = pp.tile([128, KH, G3], F8)
            s_whh1 = pp.tile([128, KH, G3], F8)
            s_fct = pp.tile([128, KH, D], F16)
            s_pred = pp.tile([128, 4, D], F16)
            s_s0 = pp.tile([B, G3], F16)      # (glob@Wg + biases) * SCALE
            s_s1 = pp.tile([B, G3], F16)      # L1 biases (r,z,n(ih)) * SCALE
            s_bh0b = pp.tile([B, H], F16)     # b_hh0 n-part bcast * SCALE
            s_bh1b = pp.tile([B, H], F16)
            s_fcb = pp.tile([D, 1], F32)
            s_pb2 = pp.tile([128, D], F32)
            s_id16 = pp.tile([B, B], F16)
            s_id16f = pp.tile([B, B], F32)
            s_id128 = pp.tile([128, 128], F16)
            # transposed states
            s_h0tm = pp.tile([128, KH, B], F32)   # masters
            s_h1tm = pp.tile([128, KH, B], F32)
            s_xh = pp.tile([128, NXT, B], F8)     # [x, 0, h0 k0..k7]
            s_h1t8 = pp.tile([128, KH, B], F8)
            s_h1t16 = pp.tile([128, KH, B], F16)  # fc operand (unscaled fp16)
            s_xt = pp.tile([128, B], F16)         # fc sigmoid out (fp16, for output row)

            # ---- init: per-queue schedule ----
            # gpsimd: embed loads -> bounce stores -> wg0 -> bcast constants
            # ACT:    fp16 converts -> globT load -> wih1 -> whh1 half
            # sync:   wcat -> state transpose-loads -> id16 -> whh1 half
            # vector: state copies/casts + s_s0 accumulation
            s_hi0 = tp.tile([B, H], F32, tag="itf32", bufs=4)
            nc.gpsimd.dma_start(out=s_hi0, in_=d_embed[:, H:2 * H])
            s_x0f = tp.tile([B, D], F32, tag="itf32", bufs=4)
            nc.gpsimd.dma_start(out=s_x0f, in_=d_x0[:, :])
            s_hi1 = tp.tile([B, H], F32, tag="itf32", bufs=4)
            nc.gpsimd.dma_start(out=s_hi1, in_=d_embed[:, 2 * H:3 * H])
            s_gf = tp.tile([B, H], F32, tag="itf32", bufs=4)
            nc.gpsimd.dma_start(out=s_gf, in_=d_embed[:, 0:H])
            s_hi0h = tp.tile([B, H], F16, tag="itf16", bufs=4)
            nc.scalar.activation(s_hi0h, s_hi0, AF.Copy)
            s_x0h = tp.tile([B, D], F16, tag="itf16", bufs=4)
            nc.scalar.activation(s_x0h, s_x0f, AF.Copy)
            s_hi1h = tp.tile([B, H], F16, tag="itf16", bufs=4)
            nc.scalar.activation(s_hi1h, s_hi1, AF.Copy)
            s_gh = tp.tile([B, H], F16, tag="itf16", bufs=4)
            nc.scalar.activation(s_gh, s_gf, AF.Copy)
            nc.gpsimd.dma_start(out=d_bh0[:, :], in_=s_hi0h)
            nc.gpsimd.dma_start(out=d_bx[:, :], in_=s_x0h)
            nc.gpsimd.dma_start(out=d_bh1[:, :], in_=s_hi1h)
            nc.gpsimd.dma_start(out=d_bg[:, :], in_=s_gh)
            s_wg = pp.tile([128, KH, G3], F8)
            nc.gpsimd.dma_start(out=s_wg, in_=d_wg0[:, :, :])
            nc.gpsimd.dma_start(out=s_s0, in_=bcast(d_s0b[:], B, G3))
            nc.gpsimd.dma_start(out=s_s1, in_=bcast(d_s1b[:], B, G3))
            nc.gpsimd.dma_start(out=s_bh0b, in_=bcast(d_bhh0n[:], B, H))
            nc.gpsimd.dma_start(out=s_bh1b, in_=bcast(d_bhh1n[:], B, H))
            nc.gpsimd.dma_start(out=s_fcb, in_=d_fcb[:, :])
            nc.gpsimd.dma_start(out=s_pb2, in_=bcast(d_pb2[:], 128, D))
            nc.gpsimd.dma_start(out=s_id128, in_=d_id128[:, :])
            nc.gpsimd.dma_start(out=s_fct, in_=d_fct[:, :, :])
            nc.gpsimd.dma_start(out=s_pred, in_=d_pred[:, :, :])

            # ACT queue: globT transpose-load, then L1 weights
            s_gT = tp.tile([128, KH, B], F16, tag="itT", bufs=5)
            nc.scalar.dma_start_transpose(s_gT[:], d_bg[:, :])
            nc.scalar.dma_start(out=s_wih1, in_=d_wih1[:, :, :])
            nc.scalar.dma_start(out=s_whh1[:, KH // 2:, :], in_=d_whh1[:, KH // 2:, :])

            # sync queue: wcat, state transpose-loads, id16, whh1 half
            nc.sync.dma_start(out=s_wcat, in_=d_wcat0[:, :, :])
            s_h0ti = tp.tile([128, KH, B], F16, tag="itT", bufs=5)
            nc.sync.dma_start_transpose(s_h0ti[:], d_bh0[:, :])
            s_xtT = tp.tile([128, 1, B], F16, tag="itT", bufs=5)
            nc.sync.dma_start_transpose(s_xtT[:], d_bx[:, :])
            s_h1ti = tp.tile([128, KH, B], F16, tag="itT", bufs=5)
            nc.sync.dma_start_transpose(s_h1ti[:], d_bh1[:, :])
            nc.sync.dma_start(out=s_id16, in_=d_id16[:, :])
            nc.sync.dma_start(out=s_whh1[:, 0:KH // 2, :], in_=d_whh1[:, 0:KH // 2, :])

            # vector queue: state init copies/casts
            nc.vector.memset(s_xh[:, 1, :], 0.0)  # zero k-slot
            nc.vector.tensor_copy(s_id16f, s_id16)
            s_gT8 = tp.tile([128, KH, B], F8, tag="itT", bufs=5)
            nc.vector.tensor_copy(s_gT8, s_gT)
            nc.vector.tensor_copy(s_h0tm, s_h0ti)
            nc.vector.tensor_copy(s_xh[:, 2:2 + KH, :], s_h0ti)
            nc.vector.tensor_copy(s_xh[:, 0, :], s_xtT.rearrange("p o b -> p (o b)"))
            nc.vector.tensor_copy(s_h1tm, s_h1ti)
            nc.vector.tensor_copy(s_h1t8, s_h1ti)
            nc.vector.tensor_copy(s_h1t16, s_h1ti)

            # s_s0 += SCALE * glob @ Wg0 (wg0 pre-scaled, fp8 DoubleRow)
            NCH = G3 // 512
            pg = [qq.tile([B, 512], F32, tag="ps", name=f"pg{c}") for c in range(NCH)]
            for c in range(NCH):
                for kp in range(KH // 2):
                    nc.tensor.matmul(pg[c], s_gT8[:, 2 * kp:2 * kp + 2, :],
                                     s_wg[:, 2 * kp:2 * kp + 2, ts(c, 512)],
                                     start=(kp == 0), stop=(kp == KH // 2 - 1),
                                     perf_mode=DR)
            for c in range(NCH):
                nc.vector.tensor_add(s_s0[:, ts(c, 512)], pg[c], s_s0[:, ts(c, 512)])

            # ---------------- one recurrence step ----------------
            # column slices for half h (h in 0,1)
            def slr(h):
                return ts(h, 512)

            def slz(h):
                return slice(H + h * 512, H + (h + 1) * 512)

            def sln(h):
                return slice(2 * H + h * 512, 2 * H + (h + 1) * 512)

            def dr_chain(p, sht8, w, colsl, start, stop, kps, soff=0):
                # fp8 DoubleRow accumulation over k-pairs; stationary pair j
                # of sht8 starts at slot soff+2j, moving pair at w[:, 2j:2j+2].
                kps = list(kps)
                for kp in kps:
                    nc.tensor.matmul(p, sht8[:, soff + 2 * kp:soff + 2 * kp + 2, :],
                                     w[:, 2 * kp:2 * kp + 2, colsl],
                                     start=(start and kp == kps[0]),
                                     stop=(stop and kp == kps[-1]), perf_mode=DR)

            def l0_h(h, tag):
                """L0 h-side chains for half h (need only old h0 in s_xh[2:10]).
                r/z psums left open (x-pair added later); gh complete."""
                p_r = qq.tile([B, 512], F32, tag="ps", name=f"{tag}r")
                dr_chain(p_r, s_xh, s_wcat[:, 2:, :], slr(h), True, False, range(4), soff=2)
                p_z = qq.tile([B, 512], F32, tag="ps", name=f"{tag}z")
                dr_chain(p_z, s_xh, s_wcat[:, 2:, :], slz(h), True, False, range(4), soff=2)
                p_gh = qq.tile([B, 512], F32, tag="ps", name=f"{tag}gh")
                dr_chain(p_gh, s_xh, s_wcat[:, 2:, :], sln(h), True, True, range(4), soff=2)
                return p_r, p_z, p_gh

            def l0_x(h, p_r, p_z):
                """x-side r/z contributions of half h (need s_xh slot 0)."""
                nc.tensor.matmul(p_r, s_xh[:, 0:2, :], s_wcat[:, 0:2, slr(h)],
                                 start=False, stop=True, perf_mode=DR)
                nc.tensor.matmul(p_z, s_xh[:, 0:2, :], s_wcat[:, 0:2, slz(h)],
                                 start=False, stop=True, perf_mode=DR)

            def l1_gh(h, tag):
                """gh1 for r,z,ghn of half h (only needs old h1t8)."""
                p_r = qq.tile([B, 512], F32, tag="ps", name=f"{tag}r")
                dr_chain(p_r, s_h1t8, s_whh1, slr(h), True, False, range(4))
                p_z = qq.tile([B, 512], F32, tag="ps", name=f"{tag}z")
                dr_chain(p_z, s_h1t8, s_whh1, slz(h), True, False, range(4))
                p_gh = qq.tile([B, 512], F32, tag="ps", name=f"{tag}gh")
                dr_chain(p_gh, s_h1t8, s_whh1, sln(h), True, True, range(4))
                return p_r, p_z, p_gh

            def l1_gi_lo(h, p_r, p_z):
                """c0@Wih1 r/z k-pairs 0-1: needs only the FIRST L0 post."""
                dr_chain(p_r, s_xh, s_wih1, slr(h), False, False, (0, 1), soff=2)
                dr_chain(p_z, s_xh, s_wih1, slz(h), False, False, (0, 1), soff=2)

            def l1_gi_hi(h, p_r, p_z, tag):
                """c0@Wih1 rest (needs the SECOND L0 post)."""
                dr_chain(p_r, s_xh, s_wih1, slr(h), False, True, (2, 3), soff=2)
                dr_chain(p_z, s_xh, s_wih1, slz(h), False, True, (2, 3), soff=2)
                p_gi = qq.tile([B, 512], F32, tag="ps", name=f"{tag}gi")
                dr_chain(p_gi, s_xh, s_wih1, sln(h), True, True, range(4), soff=2)
                return p_gi

            def post_copies(psums, engs):
                """Plain PSUM->SBUF fp16 copies (constants are folded later in
                transposed space by tiny id16 matmuls). engs: per-tile engine,
                'v' = DVE tensor_copy, 'a' = ACT activation copy."""
                cs = []
                for p, e in zip(psums, engs):
                    c = tp.tile([B, 512], F32, tag="cp", bufs=10, name="c")
                    if e == 'v':
                        nc.vector.tensor_copy(c, p)
                    else:
                        nc.scalar.activation(c, p, AF.Copy)
                    cs.append(c)
                return cs

            def post_trans(h, cs, pT, s0t, bht, gi_x):
                """PE transposes of the copied gate tiles into pT, each
                accumulation-grouped with a tiny id16 matmul that folds the
                step-invariant constant (cost = B columns, not 512). For L0
                (gi_x=True) the n-gate input side pT[:,2] is computed directly
                transposed from x (no [16,512] psum / copy / transpose).
                cs is (c_r, c_z, c_gh[, c_gi])."""
                csl = ((0, cs[0], s0t, slr(h)), (1, cs[1], s0t, slz(h)),
                       (3, cs[2], bht, ts(h, 512)))
                if not gi_x:
                    csl = csl + ((2, cs[3], s0t, sln(h)),)
                for ki, src_, const, colsl in csl:
                    cbase = colsl.start
                    for q in range(4):
                        nc.tensor.matmul(pT[:, ki, q, :],
                                         src_[:, q * 128:(q + 1) * 128], s_id16f,
                                         is_transpose=True, start=True, stop=False)
                        nc.tensor.matmul(pT[:, ki, q, :],
                                         const[:, cbase + q * 128:cbase + (q + 1) * 128],
                                         s_id16, start=False, stop=True)
                if gi_x:
                    nbase = 2 * H + h * 512
                    for q in range(4):
                        nc.tensor.matmul(pT[:, 2, q, :],
                                         s_wcat[:, 0, nbase + q * 128:nbase + (q + 1) * 128],
                                         s_xh[:, 0, :], start=True, stop=False)
                        nc.tensor.matmul(pT[:, 2, q, :],
                                         s_s0[:, nbase + q * 128:nbase + (q + 1) * 128],
                                         s_id16, start=False, stop=True)

            def post_math(h, pT, s_htm, s_ht8, extra16, sbase):
                """Transposed gate math for half h of one layer. PSUM readers
                stay on DVE/ACT; the SBUF-only update chain runs on Pool."""
                rz = tp.tile([128, 2, 4, B], F16, tag="rz", bufs=4)
                nc.scalar.activation(rz, pT[:, 0:2], AF.Sigmoid, scale=INV)
                tn = tp.tile([128, 4, B], F32, tag="tn", bufs=4)
                nc.vector.tensor_mul(tn, rz[:, 0], pT[:, 3])
                nc.vector.tensor_add(tn, tn, pT[:, 2])
                nc.scalar.activation(tn, tn, AF.Tanh, scale=INV)
                m = s_htm[:, 4 * h:4 * h + 4, :]
                td = tp.tile([128, 4, B], F32, tag="td", bufs=4)
                nc.gpsimd.tensor_sub(td, m, tn)
                nc.gpsimd.tensor_mul(td, rz[:, 1], td)
                nc.gpsimd.tensor_add(m, tn, td)
                nc.scalar.activation(s_ht8[:, sbase + 4 * h:sbase + 4 * h + 4, :],
                                     m, AF.Copy)
                if extra16 is not None:
                    nc.gpsimd.tensor_copy(extra16[:, 4 * h:4 * h + 4, :], m)

            def fc_block(t_expr, pfcT, pxo):
                for k in range(KH):
                    nc.tensor.matmul(pfcT, s_fct[:, k, :], s_h1t16[:, k, :],
                                     start=(k == 0), stop=(k == KH - 1))
                # fp8 x for the gate chain first (critical path), then fp16 row
                nc.scalar.activation(s_xh[:, 0, :], pfcT, AF.Sigmoid, bias=s_fcb[:, :])
                nc.scalar.activation(s_xt, pfcT, AF.Sigmoid, bias=s_fcb[:, :])
                nc.tensor.transpose(pxo, s_xt, s_id128)
                s_xo = tp.tile([B, D], F32, tag="xo", bufs=3, name="s_xo")
                nc.scalar.activation(s_xo, pxo, AF.Copy)
                nc.sync.dma_start(out=d_out[:, ds(t_expr, 1), :],
                                  in_=s_xo.rearrange("b d -> b () d"))

            def step(t_expr, first):
                # L0 h-side matmuls (old h0) keep the PE busy while the
                # previous step's L1 gate math finishes on the vector engines.
                r0a, z0a, gh0a = l0_h(0, "a")
                r0b, z0b, gh0b = l0_h(1, "b")
                # psum slot-rotation control: allocate before the L1 chains so
                # each tile lands on an early-released bank (see FIFO notes).
                pfcT = qq.tile([D, B], F32, tag="ps", name="pfcT")
                pxo = qq.tile([B, D], F16, tag="ps", name="pxo")
                # previous step's fc -> x (fp8 slot) + output row
                if not first:
                    fc_block(t_expr - 1, pfcT, pxo)
                l0_x(0, r0a, z0a)
                l0_x(1, r0b, z0b)
                # L0 psum->sbuf copies for BOTH halves queue up front (DVE/ACT)
                cs0a = post_copies((r0a, z0a, gh0a), ('v', 'v', 'v'))
                cs0b = post_copies((r0b, z0b, gh0b), ('a', 'v', 'v'))
                pT0a = qq.tile([128, 4, 4, B], F32, tag="ps", name="pT0a")
                pT0b = qq.tile([128, 4, 4, B], F32, tag="ps", name="pT0b")
                # L1 gh matmuls (old h1) split around the L0 transposes so the
                # PE reaches each transpose block right as its copies finish.
                r1a, z1a, gh1a = l1_gh(0, "c")
                post_trans(0, cs0a, pT0a, s_s0, s_bh0b, True)
                post_math(0, pT0a, s_h0tm, s_xh, None, 2)
                r1b, z1b, gh1b = l1_gh(1, "d")
                post_trans(1, cs0b, pT0b, s_s0, s_bh0b, True)
                post_math(1, pT0b, s_h0tm, s_xh, None, 2)
                l1_gi_lo(0, r1a, z1a)
                l1_gi_lo(1, r1b, z1b)
                gi1a = l1_gi_hi(0, r1a, z1a, "c")
                gi1b = l1_gi_hi(1, r1b, z1b, "d")
                cs1a = post_copies((r1a, z1a, gh1a, gi1a), ('v', 'a', 'v', 'a'))
                cs1b = post_copies((r1b, z1b, gh1b, gi1b), ('v', 'a', 'v', 'a'))
                pT1a = qq.tile([128, 4, 4, B], F32, tag="ps", name="pT1a")
                pT1b = qq.tile([128, 4, 4, B], F32, tag="ps", name="pT1b")
                post_trans(0, cs1a, pT1a, s_s1, s_bh1b, False)
                post_math(0, pT1a, s_h1tm, s_h1t8, s_h1t16, 0)
                post_trans(1, cs1b, pT1b, s_s1, s_bh1b, False)
                post_math(1, pT1b, s_h1tm, s_h1t8, s_h1t16, 0)

            if static:
                for t in range(n_steps):
                    step(t, t == 0)
            else:
                step(0, True)
                while (n_steps - 1) % unroll != 0:
                    unroll -= 1
                with tc.For_i(1, n_steps, unroll,
                              hint_engines=(mybir.EngineType.PE,)) as iv:
                    for j in range(unroll):
                        step(iv + j, False)
            pfcT = qq.tile([D, B], F32, tag="ps", name="pfcT")
            pxo = qq.tile([B, D], F16, tag="ps", name="pxo")
            fc_block(n_steps - 1, pfcT, pxo)

            # ---------------- tail: trend/season + residual ----------------
            for b in range(B):
                for si in range(2):
                    base = TS_OFF + si * 128 * 512
                    par = (b * 2 + si) % 2
                    ps_o = qq.tile([128, D], F32, tag="ps")
                    for which in range(2):  # 0=trend 1=season
                        off = base + which * TS_LEN
                        src = d_embed[b:b + 1, off:off + 65536].rearrange(
                            "o (s f) -> (o s) f", f=512)
                        t_f = tp.tile([128, 512], F32, tag="tsf", bufs=4)
                        deng = (nc.sync, nc.scalar)[(par + which) % 2]
                        deng.dma_start(out=t_f, in_=src)
                        t_h = tp.tile([128, 512], F16, tag="tsh", bufs=4)
                        ceng = (nc.vector, nc.gpsimd)[(par + which) % 2]
                        ceng.tensor_copy(t_h, t_f)
                        # PE transpose [128,128] chunks (f on partitions)
                        p_tT = qq.tile([128, 4, 128], F16, tag="ps", name="p_tT")
                        for jj in range(4):
                            nc.tensor.transpose(p_tT[:, jj, :],
                                                t_h[:, jj * 128:(jj + 1) * 128],
                                                s_id128)
                        t_T = tp.tile([128, 4, 128], F16, tag="tst", bufs=4)
                        teng = (nc.vector, nc.scalar)[(par + which) % 2]
                        if teng is nc.scalar:
                            teng.activation(t_T, p_tT, AF.Copy)
                        else:
                            teng.tensor_copy(t_T, p_tT)
                        for jj in range(4):
                            nc.tensor.matmul(ps_o, t_T[:, jj, :], s_pred[:, jj, :],
                                             start=(which == 0 and jj == 0),
                                             stop=(which == 1 and jj == 3))
                    r_c = tp.tile([128, D], F32, tag="rc", bufs=4)
                    (nc.sync, nc.scalar)[par].dma_start(
                        out=r_c, in_=d_out[b, si * 128:(si + 1) * 128, :])
                    nc.vector.tensor_add(r_c, ps_o, r_c)
                    nc.gpsimd.tensor_add(r_c, r_c, s_pb2)
                    (nc.scalar, nc.sync)[par].dma_start(
                        out=d_out[b, si * 128:(si + 1) * 128, :], in_=r_c)

    nc.compile()
    return nc


def _prep_weights(W_ih0, W_hh0, b_ih0, b_hh0, W_ih1, W_hh1, b_ih1, b_hh1,
                  fc_W, fc_b, pred_W, pred_b):
    f16 = np.float16
    f8 = mybir.dt.np(F8)

    def karr(WT, dt, scale=1.0):  # [K, N] -> [128, K/128, N]
        K, N = WT.shape
        return np.ascontiguousarray(
            (WT * scale).reshape(K // 128, 128, N).transpose(1, 0, 2)).astype(dt)

    wx = (np.ascontiguousarray(W_ih0[:, H:H + D].T) * SCALE).astype(f8)
    wcat = np.concatenate(
        [wx[:, None, :], np.zeros((128, 1, G3), f8), karr(W_hh0.T, f8, SCALE)],
        axis=1)

    return dict(
        wcat0=np.ascontiguousarray(wcat),
        wg0=karr(W_ih0[:, :H].T, f8, SCALE),
        wih1=karr(W_ih1.T, f8, SCALE),
        whh1=karr(W_hh1.T, f8, SCALE),
        fct=karr(fc_W.T, f16),
        predt=np.ascontiguousarray(
            pred_W.T.reshape(4, 128, D).transpose(1, 0, 2)).astype(f16),
        s0b=(np.concatenate([(b_ih0 + b_hh0)[:2 * H], b_ih0[2 * H:]]) * SCALE).astype(f16),
        s1b=(np.concatenate([(b_ih1 + b_hh1)[:2 * H], b_ih1[2 * H:]]) * SCALE).astype(f16),
        bhh0n=(b_hh0[2 * H:] * SCALE).astype(f16),
        bhh1n=(b_hh1[2 * H:] * SCALE).astype(f16),
        fcb=np.ascontiguousarray(fc_b.reshape(D, 1)).astype(np.float32),
        id16=np.eye(B, dtype=np.float16),
        id128=np.eye(128, dtype=np.float16),
        pb2=(2.0 * pred_b).astype(np.float32),
    )


_NC_CACHE = {}


def kernel(embed, dynamics, W_ih0, W_hh0, b_ih0, b_hh0,
           W_ih1, W_hh1, b_ih1, b_hh1, fc_W, fc_b, pred_W, pred_b, seq_len,
           _n_steps=SEQ, _static=False, _trace=False):
    embed = np.asarray(embed, dtype=np.float32)
    dynamics = np.asarray(dynamics, dtype=np.float32)
    wd = _prep_weights(np.asarray(W_ih0, np.float32), np.asarray(W_hh0, np.float32),
                       np.asarray(b_ih0, np.float32), np.asarray(b_hh0, np.float32),
                       np.asarray(W_ih1, np.float32), np.asarray(W_hh1, np.float32),
                       np.asarray(b_ih1, np.float32), np.asarray(b_hh1, np.float32),
                       np.asarray(fc_W, np.float32), np.asarray(fc_b, np.float32),
                       np.asarray(pred_W, np.float32), np.asarray(pred_b, np.float32))

    key = (_n_steps, _static)
    if key not in _NC_CACHE:
        _NC_CACHE[key] = build_nc(n_steps=_n_steps, static=_static)
    nc = _NC_CACHE[key]

    in_maps = []
    for c in range(NCORES):
        m = dict(wd)
        m["embed"] = np.ascontiguousarray(embed[c * B:(c + 1) * B])
        m["x0"] = np.ascontiguousarray(dynamics[c * B:(c + 1) * B, 0, :])
        in_maps.append(m)

    res = run_bass_kernel_spmd(nc, in_maps, list(range(NCORES)), trace=False)
    out = np.concatenate([res.results[c]["out"] for c in range(NCORES)], axis=0)
    if _trace:
        kernel.last_exec_time_ns = _bench_exec(nc, in_maps)
    return out


def _bench_exec(nc, in_maps, n_reps=5, k_lo=4, k_hi=20):
    """Steady-state per-execution hardware time of the sharded NEFF.

    The NTFF profiling hook is unavailable under this axon client and a
    single dispatch carries ~40-80ms of client<->terminal RPC latency, so
    a single timed call measures mostly RPC overhead. Instead dispatch
    chains of k_lo and k_hi executions asynchronously (device executions
    queue back-to-back), block once, and report the marginal time per
    execution (T(k_hi) - T(k_lo)) / (k_hi - k_lo), min over n_reps."""
    import time

    import jax
    from jax.sharding import Mesh, NamedSharding, PartitionSpec
    from jax.experimental.shard_map import shard_map

    from concourse import bass2jax, mybir as _mb

    bass2jax.install_neuronx_cc_hook()
    n_cores = len(in_maps)
    partition_name = (nc.partition_id_tensor.name if nc.partition_id_tensor else None)
    in_names, out_names, out_avals, zero_outs = [], [], [], []
    for alloc in nc.m.functions[0].allocations:
        if not isinstance(alloc, _mb.MemoryLocationSet):
            continue
        name = alloc.memorylocations[0].name
        if alloc.kind == "ExternalInput":
            if name != partition_name:
                in_names.append(name)
        elif alloc.kind == "ExternalOutput":
            out_names.append(name)
            shape = tuple(alloc.tensor_shape)
            dtype = _mb.dt.np(alloc.dtype)
            out_avals.append(jax.core.ShapedArray(shape, dtype))
            zero_outs.append(np.zeros(shape, dtype))
    n_params = len(in_names)
    all_names = list(in_names) + out_names
    if partition_name is not None:
        all_names.append(partition_name)

    def _body(*args):
        operands = list(args)
        if partition_name is not None:
            operands.append(bass2jax.partition_id_tensor())
        return tuple(bass2jax._bass_exec_p.bind(
            *operands,
            out_avals=tuple(out_avals),
            in_names=tuple(all_names),
            out_names=tuple(out_names),
            lowering_input_output_aliases=(),
            sim_require_finite=False,
            sim_require_nnan=False,
            nc=nc,
        ))

    devices = jax.devices()[:n_cores]
    mesh = Mesh(np.asarray(devices), ("core",))
    spec = PartitionSpec("core")
    fn = jax.jit(shard_map(
        _body, mesh=mesh,
        in_specs=(spec,) * (n_params + len(out_names)),
        out_specs=(spec,) * len(out_names), check_rep=False))
    sh = NamedSharding(mesh, spec)
    dev_in = [jax.device_put(
        np.concatenate([np.asarray(in_maps[c][nm]) for c in range(n_cores)], axis=0), sh)
        for nm in in_names]
    dev_zo = [jax.device_put(np.concatenate([z] * n_cores, axis=0), sh) for z in zero_outs]
    r = fn(*dev_in, *dev_zo)
    jax.block_until_ready(r)

    def chain(k):
        best = float("inf")
        for _ in range(n_reps):
            t0 = time.perf_counter()
            rs = [fn(*dev_in, *dev_zo) for _ in range(k)]
            jax.block_until_ready(rs)
            best = min(best, time.perf_counter() - t0)
        return best

    t_lo = chain(k_lo)
    t_hi = chain(k_hi)
    return int((t_hi - t_lo) / (k_hi - k_lo) * 1e9)


# revision 34
# speedup vs baseline: 1.1383x; 1.1383x over previous
"""Trainium2 Bass kernel for nn_Decoder: 2-layer GRU decoder, batch-parallel over 8 cores.

v4 design (on top of v3's transposed-gate fp8 DoubleRow formulation):
  - Shard batch 128 -> 16 rows/core, replicate weights (SBUF-resident).
  - Weight matmuls batch-major (weights moving, N=512) in fp8e4 DoubleRow;
    weights pre-scaled by SCALE=64, single scale=1/SCALE fixup inside
    sigmoid/tanh.
  - NO per-step bias/constant matmuls: the step-invariant terms
    (glob@Wg + biases, pre-scaled) are folded into the PSUM->SBUF stage,
    which becomes tensor_add instead of tensor_copy (same cost), split
    across DVE and Pool.
  - x is packed into the L0 DoubleRow chain as k-tile slot 0 of the
    augmented stationary s_xh [128, 10, B] (slot1 = zeros, slots 2-9 = h0),
    with the interleaved weight tensor wcat = [wx, 0, whh0_0..7]; the
    n-gate's input-side psum is a single (x,0) DR matmul.
  - Gate math in TRANSPOSED space (gates on 128 partitions) via PE
    transposes, elementwise update on [128, 64] tiles spread over
    DVE/Pool/ACT; hidden state produced directly in the stationary layout.
  - fp32 transposed master states; fp8 shadows for gate matmuls; fp16
    shadow of h1 for the fc head.
  - Init DMAs spread across all engine queues; tail double-buffered.
"""
import sys

sys.path.insert(0, "/opt/trn_rl_repo")
import numpy as np

import concourse.bass as bass
import concourse.mybir as mybir
import concourse.tile as tile
from concourse import bacc
from concourse.bass import ds, ts
from concourse.bass_utils import run_bass_kernel_spmd

F8 = mybir.dt.float8e4
F16 = mybir.dt.float16
F32 = mybir.dt.float32
AF = mybir.ActivationFunctionType
DR = mybir.MatmulPerfMode.DoubleRow

BS, H, D, SEQ = 128, 1024, 128, 256
NCORES = 8
B = BS // NCORES          # 16 rows per core
KH = H // 128             # 8 k-tiles over hidden dim
G3 = 3 * H                # 3072 gate cols
EMBED_DIM = 265216
TS_OFF = 3 * H
TS_LEN = SEQ * (H // 2)   # 131072
SCALE = 64.0              # fp8 weight pre-scale
INV = 1.0 / SCALE
NXT = 10                  # slots in augmented L0 stationary: x, 0, h0 k0..k7


def build_nc(n_steps=SEQ, unroll=51, static=False):
    nc = bacc.Bacc()

    d_embed = nc.declare_dram_parameter("embed", [B, EMBED_DIM], F32, isOutput=False)
    d_x0 = nc.declare_dram_parameter("x0", [B, D], F32, isOutput=False)
    d_wcat0 = nc.declare_dram_parameter("wcat0", [128, NXT, G3], F8, isOutput=False)
    d_wg0 = nc.declare_dram_parameter("wg0", [128, KH, G3], F8, isOutput=False)
    d_wih1 = nc.declare_dram_parameter("wih1", [128, KH, G3], F8, isOutput=False)
    d_whh1 = nc.declare_dram_parameter("whh1", [128, KH, G3], F8, isOutput=False)
    d_fct = nc.declare_dram_parameter("fct", [128, KH, D], F16, isOutput=False)
    d_pred = nc.declare_dram_parameter("predt", [128, 4, D], F16, isOutput=False)
    d_s0b = nc.declare_dram_parameter("s0b", [G3], F16, isOutput=False)
    d_s1b = nc.declare_dram_parameter("s1b", [G3], F16, isOutput=False)
    d_bhh0n = nc.declare_dram_parameter("bhh0n", [H], F16, isOutput=False)
    d_bhh1n = nc.declare_dram_parameter("bhh1n", [H], F16, isOutput=False)
    d_fcb = nc.declare_dram_parameter("fcb", [D, 1], F32, isOutput=False)
    d_pb2 = nc.declare_dram_parameter("pb2", [D], F32, isOutput=False)
    d_id16 = nc.declare_dram_parameter("id16", [B, B], F16, isOutput=False)
    d_id128 = nc.declare_dram_parameter("id128", [128, 128], F16, isOutput=False)
    d_out = nc.declare_dram_parameter("out", [B, SEQ, D], F32, isOutput=True)

    # DRAM scratch for init-time transposes
    d_bh0 = nc.dram_tensor("bh0", [B, H], F16)
    d_bh1 = nc.dram_tensor("bh1", [B, H], F16)
    d_bx = nc.dram_tensor("bx", [B, D], F16)
    d_bg = nc.dram_tensor("bg", [B, H], F16)

    def bcast(ap_1d, parts, n):
        return bass.AP(tensor=ap_1d.tensor, offset=ap_1d.offset,
                       ap=[[0, parts]] + list(ap_1d.ap))

    with tile.TileContext(nc) as tc:
        with (
            tc.tile_pool(name="persist", bufs=1) as pp,
            tc.tile_pool(name="tmp", bufs=2) as tp,
            tc.tile_pool(name="psum", bufs=8, space="PSUM") as qq,
        ):
            # ---------------- resident tiles ----------------
            s_wcat = pp.tile([128, NXT, G3], F8)
            s_wih1 = pp.tile([128, KH, G3], F8)
            s_whh1 = pp.tile([128, KH, G3], F8)
            s_fct = pp.tile([128, KH, D], F16)
            s_pred = pp.tile([128, 4, D], F16)
            s_s0 = pp.tile([B, G3], F16)      # (glob@Wg + biases) * SCALE
            s_s1 = pp.tile([B, G3], F16)      # L1 biases (r,z,n(ih)) * SCALE
            s_bh0b = pp.tile([B, H], F16)     # b_hh0 n-part bcast * SCALE
            s_bh1b = pp.tile([B, H], F16)
            s_fcb = pp.tile([D, 1], F32)
            s_pb2 = pp.tile([128, D], F32)
            s_id16 = pp.tile([B, B], F16)
            s_id128 = pp.tile([128, 128], F16)
            # transposed states
            s_h0tm = pp.tile([128, KH, B], F32)   # masters
            s_h1tm = pp.tile([128, KH, B], F32)
            s_xh = pp.tile([128, NXT, B], F8)     # [x, 0, h0 k0..k7]
            s_h1t8 = pp.tile([128, KH, B], F8)
            s_h1t16 = pp.tile([128, KH, B], F16)  # fc operand (unscaled fp16)
            s_xt = pp.tile([128, B], F16)         # fc sigmoid out (fp16, for output row)
            # transposed step-invariant n-gate constants [128, h, q, B]
            s_s0nT = pp.tile([128, 2, 4, B], F16)
            s_s1nT = pp.tile([128, 2, 4, B], F16)

            # ---- init: per-queue schedule ----
            # gpsimd: embed loads -> bounce stores -> wg0 -> bcast constants
            # ACT:    fp16 converts -> globT load -> wih1 -> whh1 half
            # sync:   wcat -> state transpose-loads -> id16 -> whh1 half
            # vector: state copies/casts + s_s0 accumulation
            s_hi0 = tp.tile([B, H], F32, tag="itf32", bufs=4)
            nc.gpsimd.dma_start(out=s_hi0, in_=d_embed[:, H:2 * H])
            s_x0f = tp.tile([B, D], F32, tag="itf32", bufs=4)
            nc.gpsimd.dma_start(out=s_x0f, in_=d_x0[:, :])
            s_hi1 = tp.tile([B, H], F32, tag="itf32", bufs=4)
            nc.gpsimd.dma_start(out=s_hi1, in_=d_embed[:, 2 * H:3 * H])
            s_gf = tp.tile([B, H], F32, tag="itf32", bufs=4)
            nc.gpsimd.dma_start(out=s_gf, in_=d_embed[:, 0:H])
            s_hi0h = tp.tile([B, H], F16, tag="itf16", bufs=4)
            nc.scalar.activation(s_hi0h, s_hi0, AF.Copy)
            s_x0h = tp.tile([B, D], F16, tag="itf16", bufs=4)
            nc.scalar.activation(s_x0h, s_x0f, AF.Copy)
            s_hi1h = tp.tile([B, H], F16, tag="itf16", bufs=4)
            nc.scalar.activation(s_hi1h, s_hi1, AF.Copy)
            s_gh = tp.tile([B, H], F16, tag="itf16", bufs=4)
            nc.scalar.activation(s_gh, s_gf, AF.Copy)
            nc.gpsimd.dma_start(out=d_bh0[:, :], in_=s_hi0h)
            nc.gpsimd.dma_start(out=d_bx[:, :], in_=s_x0h)
            nc.gpsimd.dma_start(out=d_bh1[:, :], in_=s_hi1h)
            nc.gpsimd.dma_start(out=d_bg[:, :], in_=s_gh)
            s_wg = pp.tile([128, KH, G3], F8)
            nc.gpsimd.dma_start(out=s_wg, in_=d_wg0[:, :, :])
            nc.gpsimd.dma_start(out=s_s0, in_=bcast(d_s0b[:], B, G3))
            nc.gpsimd.dma_start(out=s_s1, in_=bcast(d_s1b[:], B, G3))
            nc.gpsimd.dma_start(out=s_bh0b, in_=bcast(d_bhh0n[:], B, H))
            nc.gpsimd.dma_start(out=s_bh1b, in_=bcast(d_bhh1n[:], B, H))
            nc.gpsimd.dma_start(out=s_fcb, in_=d_fcb[:, :])
            nc.gpsimd.dma_start(out=s_pb2, in_=bcast(d_pb2[:], 128, D))
            nc.gpsimd.dma_start(out=s_id128, in_=d_id128[:, :])
            nc.gpsimd.dma_start(out=s_fct, in_=d_fct[:, :, :])
            nc.gpsimd.dma_start(out=s_pred, in_=d_pred[:, :, :])

            # ACT queue: globT transpose-load, then L1 weights
            s_gT = tp.tile([128, KH, B], F16, tag="itT", bufs=5)
            nc.scalar.dma_start_transpose(s_gT[:], d_bg[:, :])
            nc.scalar.dma_start(out=s_wih1, in_=d_wih1[:, :, :])
            nc.scalar.dma_start(out=s_whh1[:, KH // 2:, :], in_=d_whh1[:, KH // 2:, :])

            # sync queue: wcat, state transpose-loads, id16, whh1 half
            nc.sync.dma_start(out=s_wcat, in_=d_wcat0[:, :, :])
            s_h0ti = tp.tile([128, KH, B], F16, tag="itT", bufs=5)
            nc.sync.dma_start_transpose(s_h0ti[:], d_bh0[:, :])
            s_xtT = tp.tile([128, 1, B], F16, tag="itT", bufs=5)
            nc.sync.dma_start_transpose(s_xtT[:], d_bx[:, :])
            s_h1ti = tp.tile([128, KH, B], F16, tag="itT", bufs=5)
            nc.sync.dma_start_transpose(s_h1ti[:], d_bh1[:, :])
            nc.sync.dma_start(out=s_id16, in_=d_id16[:, :])
            nc.sync.dma_start(out=s_whh1[:, 0:KH // 2, :], in_=d_whh1[:, 0:KH // 2, :])

            # vector queue: state init copies/casts
            nc.vector.memset(s_xh[:, 1, :], 0.0)  # zero k-slot
            s_gT8 = tp.tile([128, KH, B], F8, tag="itT", bufs=5)
            nc.vector.tensor_copy(s_gT8, s_gT)
            nc.vector.tensor_copy(s_h0tm, s_h0ti)
            nc.vector.tensor_copy(s_xh[:, 2:2 + KH, :], s_h0ti)
            nc.vector.tensor_copy(s_xh[:, 0, :], s_xtT.rearrange("p o b -> p (o b)"))
            nc.vector.tensor_copy(s_h1tm, s_h1ti)
            nc.vector.tensor_copy(s_h1t8, s_h1ti)
            nc.vector.tensor_copy(s_h1t16, s_h1ti)

            # s_s0 += SCALE * glob @ Wg0 (wg0 pre-scaled, fp8 DoubleRow)
            NCH = G3 // 512
            pg = [qq.tile([B, 512], F32, tag="ps", name=f"pg{c}") for c in range(NCH)]
            for c in range(NCH):
                for kp in range(KH // 2):
                    nc.tensor.matmul(pg[c], s_gT8[:, 2 * kp:2 * kp + 2, :],
                                     s_wg[:, 2 * kp:2 * kp + 2, ts(c, 512)],
                                     start=(kp == 0), stop=(kp == KH // 2 - 1),
                                     perf_mode=DR)
            for c in range(NCH):
                nc.vector.tensor_add(s_s0[:, ts(c, 512)], pg[c], s_s0[:, ts(c, 512)])

            # transposed n-column constants for both layers
            for src_, dnT in ((s_s0, s_s0nT), (s_s1, s_s1nT)):
                pzn = qq.tile([128, 2, 4, B], F16, tag="ps", name="pzn")
                for hh in range(2):
                    for q in range(4):
                        col = 2 * H + hh * 512 + q * 128
                        nc.tensor.transpose(pzn[:, hh, q, :],
                                            src_[:, col:col + 128], s_id16)
                nc.vector.tensor_copy(dnT, pzn)

            # ---------------- one recurrence step ----------------
            # column slices for half h (h in 0,1)
            def slr(h):
                return ts(h, 512)

            def slz(h):
                return slice(H + h * 512, H + (h + 1) * 512)

            def sln(h):
                return slice(2 * H + h * 512, 2 * H + (h + 1) * 512)

            def dr_chain(p, sht8, w, colsl, start, stop, kps, soff=0):
                # fp8 DoubleRow accumulation over k-pairs; stationary pair j
                # of sht8 starts at slot soff+2j, moving pair at w[:, 2j:2j+2].
                kps = list(kps)
                for kp in kps:
                    nc.tensor.matmul(p, sht8[:, soff + 2 * kp:soff + 2 * kp + 2, :],
                                     w[:, 2 * kp:2 * kp + 2, colsl],
                                     start=(start and kp == kps[0]),
                                     stop=(stop and kp == kps[-1]), perf_mode=DR)

            def l0_h(h, tag):
                """L0 h-side chains for half h (need only old h0 in s_xh[2:10]).
                r/z psums left open (x-pair added later); gh complete."""
                p_r = qq.tile([B, 512], F32, tag="ps", name=f"{tag}r")
                dr_chain(p_r, s_xh, s_wcat[:, 2:, :], slr(h), True, False, range(4), soff=2)
                p_z = qq.tile([B, 512], F32, tag="ps", name=f"{tag}z")
                dr_chain(p_z, s_xh, s_wcat[:, 2:, :], slz(h), True, False, range(4), soff=2)
                p_gh = qq.tile([B, 512], F32, tag="ps", name=f"{tag}gh")
                dr_chain(p_gh, s_xh, s_wcat[:, 2:, :], sln(h), True, True, range(4), soff=2)
                return p_r, p_z, p_gh

            def l0_x(h, p_r, p_z, tag):
                """x-side contributions of half h (need s_xh slot 0)."""
                nc.tensor.matmul(p_r, s_xh[:, 0:2, :], s_wcat[:, 0:2, slr(h)],
                                 start=False, stop=True, perf_mode=DR)
                nc.tensor.matmul(p_z, s_xh[:, 0:2, :], s_wcat[:, 0:2, slz(h)],
                                 start=False, stop=True, perf_mode=DR)
                p_gi = qq.tile([B, 512], F32, tag="ps", name=f"{tag}gi")
                nc.tensor.matmul(p_gi, s_xh[:, 0:2, :], s_wcat[:, 0:2, sln(h)],
                                 start=True, stop=True, perf_mode=DR)
                return p_gi

            def l1_gh(h, tag):
                """gh1 for r,z,ghn of half h (only needs old h1t8)."""
                p_r = qq.tile([B, 512], F32, tag="ps", name=f"{tag}r")
                dr_chain(p_r, s_h1t8, s_whh1, slr(h), True, False, range(4))
                p_z = qq.tile([B, 512], F32, tag="ps", name=f"{tag}z")
                dr_chain(p_z, s_h1t8, s_whh1, slz(h), True, False, range(4))
                p_gh = qq.tile([B, 512], F32, tag="ps", name=f"{tag}gh")
                dr_chain(p_gh, s_h1t8, s_whh1, sln(h), True, True, range(4))
                return p_r, p_z, p_gh

            def l1_gi_lo(h, p_r, p_z):
                """c0@Wih1 r/z k-pairs 0-1: needs only the FIRST L0 post."""
                dr_chain(p_r, s_xh, s_wih1, slr(h), False, False, (0, 1), soff=2)
                dr_chain(p_z, s_xh, s_wih1, slz(h), False, False, (0, 1), soff=2)

            def l1_gi_hi(h, p_r, p_z, tag):
                """c0@Wih1 rest (needs the SECOND L0 post)."""
                dr_chain(p_r, s_xh, s_wih1, slr(h), False, True, (2, 3), soff=2)
                dr_chain(p_z, s_xh, s_wih1, slz(h), False, True, (2, 3), soff=2)
                p_gi = qq.tile([B, 512], F32, tag="ps", name=f"{tag}gi")
                dr_chain(p_gi, s_xh, s_wih1, sln(h), True, True, range(4), soff=2)
                return p_gi

            def post_adds(h, p_r, p_z, p_gi, p_gh, s0t, bht, three_dve):
                """PSUM->SBUF stage. Only DVE (tensor_add, folds a constant
                for free) and ACT (plain copy) can read PSUM on real HW; the
                constants for ACT-copied tiles are folded later in transposed
                space (z: DVE on psum pT, gi: Pool on SBUF tn)."""
                c_r = tp.tile([B, 512], F16, tag="cp", bufs=8, name="c_r")
                c_z = tp.tile([B, 512], F16, tag="cp", bufs=8, name="c_z")
                c_gi = tp.tile([B, 512], F16, tag="cp", bufs=8, name="c_gi")
                c_gh = tp.tile([B, 512], F16, tag="cp", bufs=8, name="c_gh")
                nc.vector.tensor_add(c_r, p_r, s0t[:, slr(h)])
                nc.scalar.activation(c_gi, p_gi, AF.Copy)
                if three_dve:
                    nc.vector.tensor_add(c_z, p_z, s0t[:, slz(h)])
                else:
                    nc.scalar.activation(c_z, p_z, AF.Copy)
                    nc.gpsimd.tensor_add(c_z, c_z, s0t[:, slz(h)])
                nc.vector.tensor_add(c_gh, p_gh, bht[:, ts(h, 512)])
                return (c_r, c_z, c_gi, c_gh)

            def post_trans(cs, pT):
                for ki, src_ in enumerate(cs):
                    for q in range(4):
                        nc.tensor.transpose(pT[:, ki, q, :],
                                            src_[:, q * 128:(q + 1) * 128], s_id16)

            def post_math(h, pT, s0nT, s_htm, s_ht8, extra16, sbase):
                """Transposed gate math for half h of one layer. PSUM readers
                stay on DVE/ACT; the SBUF-only update chain runs on Pool."""
                rz = tp.tile([128, 2, 4, B], F16, tag="rz", bufs=4)
                nc.scalar.activation(rz, pT[:, 0:2], AF.Sigmoid, scale=INV)
                tn = tp.tile([128, 4, B], F32, tag="tn", bufs=4)
                nc.vector.tensor_mul(tn, rz[:, 0], pT[:, 3])
                nc.vector.tensor_add(tn, tn, pT[:, 2])
                nc.gpsimd.tensor_add(tn, tn, s0nT[:, h])
                nc.scalar.activation(tn, tn, AF.Tanh, scale=INV)
                m = s_htm[:, 4 * h:4 * h + 4, :]
                td = tp.tile([128, 4, B], F32, tag="td", bufs=4)
                nc.gpsimd.tensor_sub(td, m, tn)
                nc.gpsimd.tensor_mul(td, rz[:, 1], td)
                nc.gpsimd.tensor_add(m, tn, td)
                nc.scalar.activation(s_ht8[:, sbase + 4 * h:sbase + 4 * h + 4, :],
                                     m, AF.Copy)
                if extra16 is not None:
                    nc.gpsimd.tensor_copy(extra16[:, 4 * h:4 * h + 4, :], m)

            def fc_block(t_expr, pfcT, pxo):
                for k in range(KH):
                    nc.tensor.matmul(pfcT, s_fct[:, k, :], s_h1t16[:, k, :],
                                     start=(k == 0), stop=(k == KH - 1))
                # fp8 x for the gate chain first (critical path), then fp16 row
                nc.scalar.activation(s_xh[:, 0, :], pfcT, AF.Sigmoid, bias=s_fcb[:, :])
                nc.scalar.activation(s_xt, pfcT, AF.Sigmoid, bias=s_fcb[:, :])
                nc.tensor.transpose(pxo, s_xt, s_id128)
                s_xo = tp.tile([B, D], F32, tag="xo", bufs=3, name="s_xo")
                nc.scalar.activation(s_xo, pxo, AF.Copy)
                nc.sync.dma_start(out=d_out[:, ds(t_expr, 1), :],
                                  in_=s_xo.rearrange("b d -> b () d"))

            def step(t_expr, first):
                # L0 h-side matmuls (old h0) keep the PE busy while the
                # previous step's L1 gate math finishes on the vector engines.
                r0a, z0a, gh0a = l0_h(0, "a")
                r0b, z0b, gh0b = l0_h(1, "b")
                # psum slot-rotation control: allocate before the L1 chains so
                # each tile lands on an early-released bank (see FIFO notes).
                pfcT = qq.tile([D, B], F32, tag="ps", name="pfcT")
                pxo = qq.tile([B, D], F16, tag="ps", name="pxo")
                # previous step's fc -> x (fp8 slot) + output row
                if not first:
                    fc_block(t_expr - 1, pfcT, pxo)
                gi0a = l0_x(0, r0a, z0a, "a")
                gi0b = l0_x(1, r0b, z0b, "b")
                # L0 psum->sbuf stage for BOTH halves queues up front (DVE/ACT)
                cs0a = post_adds(0, r0a, z0a, gi0a, gh0a, s_s0, s_bh0b, True)
                cs0b = post_adds(1, r0b, z0b, gi0b, gh0b, s_s0, s_bh0b, False)
                pT0a = qq.tile([128, 4, 4, B], F16, tag="ps", name="pT0a")
                pT0b = qq.tile([128, 4, 4, B], F16, tag="ps", name="pT0b")
                # L1 gh matmuls (old h1) split around the L0 transposes so the
                # PE reaches each transpose block right as its adds finish.
                r1a, z1a, gh1a = l1_gh(0, "c")
                post_trans(cs0a, pT0a)
                post_math(0, pT0a, s_s0nT, s_h0tm, s_xh, None, 2)
                r1b, z1b, gh1b = l1_gh(1, "d")
                post_trans(cs0b, pT0b)
                post_math(1, pT0b, s_s0nT, s_h0tm, s_xh, None, 2)
                l1_gi_lo(0, r1a, z1a)
                l1_gi_lo(1, r1b, z1b)
                gi1a = l1_gi_hi(0, r1a, z1a, "c")
                gi1b = l1_gi_hi(1, r1b, z1b, "d")
                cs1a = post_adds(0, r1a, z1a, gi1a, gh1a, s_s1, s_bh1b, True)
                cs1b = post_adds(1, r1b, z1b, gi1b, gh1b, s_s1, s_bh1b, False)
                pT1a = qq.tile([128, 4, 4, B], F16, tag="ps", name="pT1a")
                pT1b = qq.tile([128, 4, 4, B], F16, tag="ps", name="pT1b")
                post_trans(cs1a, pT1a)
                post_math(0, pT1a, s_s1nT, s_h1tm, s_h1t8, s_h1t16, 0)
                post_trans(cs1b, pT1b)
                post_math(1, pT1b, s_s1nT, s_h1tm, s_h1t8, s_h1t16, 0)

            if static:
                for t in range(n_steps):
                    step(t, t == 0)
            else:
                step(0, True)
                while (n_steps - 1) % unroll != 0:
                    unroll -= 1
                with tc.For_i(1, n_steps, unroll,
                              hint_engines=(mybir.EngineType.PE,)) as iv:
                    for j in range(unroll):
                        step(iv + j, False)
            pfcT = qq.tile([D, B], F32, tag="ps", name="pfcT")
            pxo = qq.tile([B, D], F16, tag="ps", name="pxo")
            fc_block(n_steps - 1, pfcT, pxo)

            # ---------------- tail: trend/season + residual ----------------
            for b in range(B):
                for si in range(2):
                    base = TS_OFF + si * 128 * 512
                    par = (b * 2 + si) % 2
                    ps_o = qq.tile([128, D], F32, tag="ps")
                    for which in range(2):  # 0=trend 1=season
                        off = base + which * TS_LEN
                        src = d_embed[b:b + 1, off:off + 65536].rearrange(
                            "o (s f) -> (o s) f", f=512)
                        t_f = tp.tile([128, 512], F32, tag="tsf", bufs=4)
                        deng = (nc.sync, nc.scalar)[(par + which) % 2]
                        deng.dma_start(out=t_f, in_=src)
                        t_h = tp.tile([128, 512], F16, tag="tsh", bufs=4)
                        ceng = (nc.vector, nc.gpsimd)[(par + which) % 2]
                        ceng.tensor_copy(t_h, t_f)
                        # PE transpose [128,128] chunks (f on partitions)
                        p_tT = qq.tile([128, 4, 128], F16, tag="ps", name="p_tT")
                        for jj in range(4):
                            nc.tensor.transpose(p_tT[:, jj, :],
                                                t_h[:, jj * 128:(jj + 1) * 128],
                                                s_id128)
                        t_T = tp.tile([128, 4, 128], F16, tag="tst", bufs=4)
                        teng = (nc.vector, nc.scalar)[(par + which) % 2]
                        if teng is nc.scalar:
                            teng.activation(t_T, p_tT, AF.Copy)
                        else:
                            teng.tensor_copy(t_T, p_tT)
                        for jj in range(4):
                            nc.tensor.matmul(ps_o, t_T[:, jj, :], s_pred[:, jj, :],
                                             start=(which == 0 and jj == 0),
                                             stop=(which == 1 and jj == 3))
                    r_c = tp.tile([128, D], F32, tag="rc", bufs=4)
                    (nc.sync, nc.scalar)[par].dma_start(
                        out=r_c, in_=d_out[b, si * 128:(si + 1) * 128, :])
                    nc.vector.tensor_add(r_c, ps_o, r_c)
                    nc.gpsimd.tensor_add(r_c, r_c, s_pb2)
                    (nc.scalar, nc.sync)[par].dma_start(
                        out=d_out[b, si * 128:(si + 1) * 128, :], in_=r_c)

    nc.compile()
    return nc


def _prep_weights(W_ih0, W_hh0, b_ih0, b_hh0, W_ih1, W_hh1, b_ih1, b_hh1,
                  fc_W, fc_b, pred_W, pred_b):
    f16 = np.float16
    f8 = mybir.dt.np(F8)

    def karr(WT, dt, scale=1.0):  # [K, N] -> [128, K/128, N]
        K, N = WT.shape
        return np.ascontiguousarray(
            (WT * scale).reshape(K // 128, 128, N).transpose(1, 0, 2)).astype(dt)

    wx = (np.ascontiguousarray(W_ih0[:, H:H + D].T) * SCALE).astype(f8)
    wcat = np.concatenate(
        [wx[:, None, :], np.zeros((128, 1, G3), f8), karr(W_hh0.T, f8, SCALE)],
        axis=1)

    return dict(
        wcat0=np.ascontiguousarray(wcat),
        wg0=karr(W_ih0[:, :H].T, f8, SCALE),
        wih1=karr(W_ih1.T, f8, SCALE),
        whh1=karr(W_hh1.T, f8, SCALE),
        fct=karr(fc_W.T, f16),
        predt=np.ascontiguousarray(
            pred_W.T.reshape(4, 128, D).transpose(1, 0, 2)).astype(f16),
        s0b=(np.concatenate([(b_ih0 + b_hh0)[:2 * H], b_ih0[2 * H:]]) * SCALE).astype(f16),
        s1b=(np.concatenate([(b_ih1 + b_hh1)[:2 * H], b_ih1[2 * H:]]) * SCALE).astype(f16),
        bhh0n=(b_hh0[2 * H:] * SCALE).astype(f16),
        bhh1n=(b_hh1[2 * H:] * SCALE).astype(f16),
        fcb=np.ascontiguousarray(fc_b.reshape(D, 1)).astype(np.float32),
        id16=np.eye(B, dtype=np.float16),
        id128=np.eye(128, dtype=np.float16),
        pb2=(2.0 * pred_b).astype(np.float32),
    )


_NC_CACHE = {}


def kernel(embed, dynamics, W_ih0, W_hh0, b_ih0, b_hh0,
           W_ih1, W_hh1, b_ih1, b_hh1, fc_W, fc_b, pred_W, pred_b, seq_len,
           _n_steps=SEQ, _static=False, _trace=False):
    embed = np.asarray(embed, dtype=np.float32)
    dynamics = np.asarray(dynamics, dtype=np.float32)
    wd = _prep_weights(np.asarray(W_ih0, np.float32), np.asarray(W_hh0, np.float32),
                       np.asarray(b_ih0, np.float32), np.asarray(b_hh0, np.float32),
                       np.asarray(W_ih1, np.float32), np.asarray(W_hh1, np.float32),
                       np.asarray(b_ih1, np.float32), np.asarray(b_hh1, np.float32),
                       np.asarray(fc_W, np.float32), np.asarray(fc_b, np.float32),
                       np.asarray(pred_W, np.float32), np.asarray(pred_b, np.float32))

    key = (_n_steps, _static)
    if key not in _NC_CACHE:
        _NC_CACHE[key] = build_nc(n_steps=_n_steps, static=_static)
    nc = _NC_CACHE[key]

    in_maps = []
    for c in range(NCORES):
        m = dict(wd)
        m["embed"] = np.ascontiguousarray(embed[c * B:(c + 1) * B])
        m["x0"] = np.ascontiguousarray(dynamics[c * B:(c + 1) * B, 0, :])
        in_maps.append(m)

    res = run_bass_kernel_spmd(nc, in_maps, list(range(NCORES)), trace=False)
    out = np.concatenate([res.results[c]["out"] for c in range(NCORES)], axis=0)
    if _trace:
        kernel.last_exec_time_ns = _bench_exec(nc, in_maps)
    return out


def _bench_exec(nc, in_maps, n_reps=5, k_lo=4, k_hi=20):
    """Steady-state per-execution hardware time of the sharded NEFF.

    The NTFF profiling hook is unavailable under this axon client and a
    single dispatch carries ~40-80ms of client<->terminal RPC latency, so
    a single timed call measures mostly RPC overhead. Instead dispatch
    chains of k_lo and k_hi executions asynchronously (device executions
    queue back-to-back), block once, and report the marginal time per
    execution (T(k_hi) - T(k_lo)) / (k_hi - k_lo), min over n_reps."""
    import time

    import jax
    from jax.sharding import Mesh, NamedSharding, PartitionSpec
    from jax.experimental.shard_map import shard_map

    from concourse import bass2jax, mybir as _mb

    bass2jax.install_neuronx_cc_hook()
    n_cores = len(in_maps)
    partition_name = (nc.partition_id_tensor.name if nc.partition_id_tensor else None)
    in_names, out_names, out_avals, zero_outs = [], [], [], []
    for alloc in nc.m.functions[0].allocations:
        if not isinstance(alloc, _mb.MemoryLocationSet):
            continue
        name = alloc.memorylocations[0].name
        if alloc.kind == "ExternalInput":
            if name != partition_name:
                in_names.append(name)
        elif alloc.kind == "ExternalOutput":
            out_names.append(name)
            shape = tuple(alloc.tensor_shape)
            dtype = _mb.dt.np(alloc.dtype)
            out_avals.append(jax.core.ShapedArray(shape, dtype))
            zero_outs.append(np.zeros(shape, dtype))
    n_params = len(in_names)
    all_names = list(in_names) + out_names
    if partition_name is not None:
        all_names.append(partition_name)

    def _body(*args):
        operands = list(args)
        if partition_name is not None:
            operands.append(bass2jax.partition_id_tensor())
        return tuple(bass2jax._bass_exec_p.bind(
            *operands,
            out_avals=tuple(out_avals),
            in_names=tuple(all_names),
            out_names=tuple(out_names),
            lowering_input_output_aliases=(),
            sim_require_finite=False,
            sim_require_nnan=False,
            nc=nc,
        ))

    devices = jax.devices()[:n_cores]
    mesh = Mesh(np.asarray(devices), ("core",))
    spec = PartitionSpec("core")
    fn = jax.jit(shard_map(
        _body, mesh=mesh,
        in_specs=(spec,) * (n_params + len(out_names)),
        out_specs=(spec,) * len(out_names), check_rep=False))
    sh = NamedSharding(mesh, spec)
    dev_in = [jax.device_put(
        np.concatenate([np.asarray(in_maps[c][nm]) for c in range(n_cores)], axis=0), sh)
        for nm in in_names]
    dev_zo = [jax.device_put(np.concatenate([z] * n_cores, axis=0), sh) for z in zero_outs]
    r = fn(*dev_in, *dev_zo)
    jax.block_until_ready(r)

    def chain(k):
        best = float("inf")
        for _ in range(n_reps):
            t0 = time.perf_counter()
            rs = [fn(*dev_in, *dev_zo) for _ in range(k)]
            jax.block_until_ready(rs)
            best = min(best, time.perf_counter() - t0)
        return best

    t_lo = chain(k_lo)
    t_hi = chain(k_hi)
    return int((t_hi - t_lo) / (k_hi - k_lo) * 1e9)


# revision 37
# speedup vs baseline: 1.1796x; 1.0363x over previous
"""Trainium2 Bass kernel for nn_Decoder: 2-layer GRU decoder, batch-parallel over 8 cores.

v4 design (on top of v3's transposed-gate fp8 DoubleRow formulation):
  - Shard batch 128 -> 16 rows/core, replicate weights (SBUF-resident).
  - Weight matmuls batch-major (weights moving, N=512) in fp8e4 DoubleRow;
    weights pre-scaled by SCALE=64, single scale=1/SCALE fixup inside
    sigmoid/tanh.
  - NO per-step bias/constant matmuls: the step-invariant terms
    (glob@Wg + biases, pre-scaled) are folded into the PSUM->SBUF stage,
    which becomes tensor_add instead of tensor_copy (same cost), split
    across DVE and Pool.
  - x is packed into the L0 DoubleRow chain as k-tile slot 0 of the
    augmented stationary s_xh [128, 10, B] (slot1 = zeros, slots 2-9 = h0),
    with the interleaved weight tensor wcat = [wx, 0, whh0_0..7]; the
    n-gate's input-side psum is a single (x,0) DR matmul.
  - Gate math in TRANSPOSED space (gates on 128 partitions) via PE
    transposes, elementwise update on [128, 64] tiles spread over
    DVE/Pool/ACT; hidden state produced directly in the stationary layout.
  - fp32 transposed master states; fp8 shadows for gate matmuls; fp16
    shadow of h1 for the fc head.
  - Init DMAs spread across all engine queues; tail double-buffered.
"""
import sys

sys.path.insert(0, "/opt/trn_rl_repo")
import numpy as np

import concourse.bass as bass
import concourse.mybir as mybir
import concourse.tile as tile
from concourse import bacc
from concourse.bass import ds, ts
from concourse.bass_utils import run_bass_kernel_spmd

F8 = mybir.dt.float8e4
F16 = mybir.dt.float16
F32 = mybir.dt.float32
AF = mybir.ActivationFunctionType
DR = mybir.MatmulPerfMode.DoubleRow

BS, H, D, SEQ = 128, 1024, 128, 256
NCORES = 8
B = BS // NCORES          # 16 rows per core
KH = H // 128             # 8 k-tiles over hidden dim
G3 = 3 * H                # 3072 gate cols
EMBED_DIM = 265216
TS_OFF = 3 * H
TS_LEN = SEQ * (H // 2)   # 131072
SCALE = 64.0              # fp8 weight pre-scale
INV = 1.0 / SCALE
NXT = 10                  # slots in augmented L0 stationary: x, 0, h0 k0..k7


def build_nc(n_steps=SEQ, unroll=51, static=False):
    nc = bacc.Bacc()

    d_embed = nc.declare_dram_parameter("embed", [B, EMBED_DIM], F32, isOutput=False)
    d_x0 = nc.declare_dram_parameter("x0", [B, D], F32, isOutput=False)
    d_wcat0 = nc.declare_dram_parameter("wcat0", [128, NXT, G3], F8, isOutput=False)
    d_wg0 = nc.declare_dram_parameter("wg0", [128, KH, G3], F8, isOutput=False)
    d_wih1 = nc.declare_dram_parameter("wih1", [128, KH, G3], F8, isOutput=False)
    d_whh1 = nc.declare_dram_parameter("whh1", [128, KH, G3], F8, isOutput=False)
    d_fct = nc.declare_dram_parameter("fct", [128, KH, D], F16, isOutput=False)
    d_pred = nc.declare_dram_parameter("predt", [128, 4, D], F16, isOutput=False)
    d_s0b = nc.declare_dram_parameter("s0b", [G3], F16, isOutput=False)
    d_s1b = nc.declare_dram_parameter("s1b", [G3], F16, isOutput=False)
    d_bhh0n = nc.declare_dram_parameter("bhh0n", [H], F16, isOutput=False)
    d_bhh1n = nc.declare_dram_parameter("bhh1n", [H], F16, isOutput=False)
    d_fcb = nc.declare_dram_parameter("fcb", [D, 1], F32, isOutput=False)
    d_pb2 = nc.declare_dram_parameter("pb2", [D], F32, isOutput=False)
    d_id16 = nc.declare_dram_parameter("id16", [B, B], F16, isOutput=False)
    d_id128 = nc.declare_dram_parameter("id128", [128, 128], F16, isOutput=False)
    d_out = nc.declare_dram_parameter("out", [B, SEQ, D], F32, isOutput=True)

    # DRAM scratch for init-time transposes
    d_bh0 = nc.dram_tensor("bh0", [B, H], F16)
    d_bh1 = nc.dram_tensor("bh1", [B, H], F16)
    d_bx = nc.dram_tensor("bx", [B, D], F16)
    d_bg = nc.dram_tensor("bg", [B, H], F16)

    def bcast(ap_1d, parts, n):
        return bass.AP(tensor=ap_1d.tensor, offset=ap_1d.offset,
                       ap=[[0, parts]] + list(ap_1d.ap))

    with tile.TileContext(nc) as tc:
        with (
            tc.tile_pool(name="persist", bufs=1) as pp,
            tc.tile_pool(name="tmp", bufs=2) as tp,
            tc.tile_pool(name="psum", bufs=8, space="PSUM") as qq,
        ):
            # ---------------- resident tiles ----------------
            s_wcat = pp.tile([128, NXT, G3], F8)
            s_wih1 = pp.tile([128, KH, G3], F8)
            s_whh1 = pp.tile([128, KH, G3], F8)
            s_fct = pp.tile([128, KH, D], F16)
            s_pred = pp.tile([128, 4, D], F16)
            s_s0 = pp.tile([B, G3], F16)      # (glob@Wg + biases) * SCALE
            s_s1 = pp.tile([B, G3], F16)      # L1 biases (r,z,n(ih)) * SCALE
            s_bh0b = pp.tile([B, H], F16)     # b_hh0 n-part bcast * SCALE
            s_bh1b = pp.tile([B, H], F16)
            s_fcb = pp.tile([D, 1], F32)
            s_pb2 = pp.tile([128, D], F32)
            s_id16 = pp.tile([B, B], F16)
            s_id128 = pp.tile([128, 128], F16)
            # transposed states
            s_h0tm = pp.tile([128, KH, B], F32)   # masters
            s_h1tm = pp.tile([128, KH, B], F32)
            s_xh = pp.tile([128, NXT, B], F8)     # [x, 0, h0 k0..k7]
            s_h1t8 = pp.tile([128, KH, B], F8)
            s_h1t16 = pp.tile([128, KH, B], F16)  # fc operand (unscaled fp16)
            s_xt = pp.tile([128, B], F16)         # fc sigmoid out (fp16, for output row)
            # transposed step-invariant n-gate constants [128, h, q, B]
            s_s0nT = pp.tile([128, 2, 4, B], F16)
            s_s1nT = pp.tile([128, 2, 4, B], F16)

            # ---- init: per-queue schedule ----
            # gpsimd: embed loads -> bounce stores -> wg0 -> bcast constants
            # ACT:    fp16 converts -> globT load -> wih1 -> whh1 half
            # sync:   wcat -> state transpose-loads -> id16 -> whh1 half
            # vector: state copies/casts + s_s0 accumulation
            s_hi0 = tp.tile([B, H], F32, tag="itf32", bufs=4)
            nc.gpsimd.dma_start(out=s_hi0, in_=d_embed[:, H:2 * H])
            s_x0f = tp.tile([B, D], F32, tag="itf32", bufs=4)
            nc.gpsimd.dma_start(out=s_x0f, in_=d_x0[:, :])
            s_hi1 = tp.tile([B, H], F32, tag="itf32", bufs=4)
            nc.gpsimd.dma_start(out=s_hi1, in_=d_embed[:, 2 * H:3 * H])
            s_gf = tp.tile([B, H], F32, tag="itf32", bufs=4)
            nc.gpsimd.dma_start(out=s_gf, in_=d_embed[:, 0:H])
            s_hi0h = tp.tile([B, H], F16, tag="itf16", bufs=4)
            nc.scalar.activation(s_hi0h, s_hi0, AF.Copy)
            s_x0h = tp.tile([B, D], F16, tag="itf16", bufs=4)
            nc.scalar.activation(s_x0h, s_x0f, AF.Copy)
            s_hi1h = tp.tile([B, H], F16, tag="itf16", bufs=4)
            nc.scalar.activation(s_hi1h, s_hi1, AF.Copy)
            s_gh = tp.tile([B, H], F16, tag="itf16", bufs=4)
            nc.scalar.activation(s_gh, s_gf, AF.Copy)
            nc.gpsimd.dma_start(out=d_bh0[:, :], in_=s_hi0h)
            nc.gpsimd.dma_start(out=d_bx[:, :], in_=s_x0h)
            nc.gpsimd.dma_start(out=d_bh1[:, :], in_=s_hi1h)
            nc.gpsimd.dma_start(out=d_bg[:, :], in_=s_gh)
            s_wg = pp.tile([128, KH, G3], F8)
            nc.gpsimd.dma_start(out=s_wg, in_=d_wg0[:, :, :])
            nc.gpsimd.dma_start(out=s_s0, in_=bcast(d_s0b[:], B, G3))
            nc.gpsimd.dma_start(out=s_s1, in_=bcast(d_s1b[:], B, G3))
            nc.gpsimd.dma_start(out=s_bh0b, in_=bcast(d_bhh0n[:], B, H))
            nc.gpsimd.dma_start(out=s_bh1b, in_=bcast(d_bhh1n[:], B, H))
            nc.gpsimd.dma_start(out=s_fcb, in_=d_fcb[:, :])
            nc.gpsimd.dma_start(out=s_pb2, in_=bcast(d_pb2[:], 128, D))
            nc.gpsimd.dma_start(out=s_id128, in_=d_id128[:, :])
            nc.gpsimd.dma_start(out=s_fct, in_=d_fct[:, :, :])
            nc.gpsimd.dma_start(out=s_pred, in_=d_pred[:, :, :])

            # ACT queue: globT transpose-load, then L1 weights
            s_gT = tp.tile([128, KH, B], F16, tag="itT", bufs=5)
            nc.scalar.dma_start_transpose(s_gT[:], d_bg[:, :])
            nc.scalar.dma_start(out=s_wih1, in_=d_wih1[:, :, :])
            nc.scalar.dma_start(out=s_whh1[:, KH // 2:, :], in_=d_whh1[:, KH // 2:, :])

            # sync queue: wcat, state transpose-loads, id16, whh1 half
            nc.sync.dma_start(out=s_wcat, in_=d_wcat0[:, :, :])
            s_h0ti = tp.tile([128, KH, B], F16, tag="itT", bufs=5)
            nc.sync.dma_start_transpose(s_h0ti[:], d_bh0[:, :])
            s_xtT = tp.tile([128, 1, B], F16, tag="itT", bufs=5)
            nc.sync.dma_start_transpose(s_xtT[:], d_bx[:, :])
            s_h1ti = tp.tile([128, KH, B], F16, tag="itT", bufs=5)
            nc.sync.dma_start_transpose(s_h1ti[:], d_bh1[:, :])
            nc.sync.dma_start(out=s_id16, in_=d_id16[:, :])
            nc.sync.dma_start(out=s_whh1[:, 0:KH // 2, :], in_=d_whh1[:, 0:KH // 2, :])

            # vector queue: state init copies/casts
            nc.vector.memset(s_xh[:, 1, :], 0.0)  # zero k-slot
            s_gT8 = tp.tile([128, KH, B], F8, tag="itT", bufs=5)
            nc.vector.tensor_copy(s_gT8, s_gT)
            nc.vector.tensor_copy(s_h0tm, s_h0ti)
            nc.vector.tensor_copy(s_xh[:, 2:2 + KH, :], s_h0ti)
            nc.vector.tensor_copy(s_xh[:, 0, :], s_xtT.rearrange("p o b -> p (o b)"))
            nc.vector.tensor_copy(s_h1tm, s_h1ti)
            nc.vector.tensor_copy(s_h1t8, s_h1ti)
            nc.vector.tensor_copy(s_h1t16, s_h1ti)

            # s_s0 += SCALE * glob @ Wg0 (wg0 pre-scaled, fp8 DoubleRow)
            NCH = G3 // 512
            pg = [qq.tile([B, 512], F32, tag="ps", name=f"pg{c}") for c in range(NCH)]
            for c in range(NCH):
                for kp in range(KH // 2):
                    nc.tensor.matmul(pg[c], s_gT8[:, 2 * kp:2 * kp + 2, :],
                                     s_wg[:, 2 * kp:2 * kp + 2, ts(c, 512)],
                                     start=(kp == 0), stop=(kp == KH // 2 - 1),
                                     perf_mode=DR)
            for c in range(NCH):
                nc.vector.tensor_add(s_s0[:, ts(c, 512)], pg[c], s_s0[:, ts(c, 512)])

            # transposed n-column constants for both layers
            for src_, dnT in ((s_s0, s_s0nT), (s_s1, s_s1nT)):
                pzn = qq.tile([128, 2, 4, B], F16, tag="ps", name="pzn")
                for hh in range(2):
                    for q in range(4):
                        col = 2 * H + hh * 512 + q * 128
                        nc.tensor.transpose(pzn[:, hh, q, :],
                                            src_[:, col:col + 128], s_id16)
                nc.vector.tensor_copy(dnT, pzn)

            # ---------------- one recurrence step ----------------
            # column slices for half h (h in 0,1)
            def slr(h):
                return ts(h, 512)

            def slz(h):
                return slice(H + h * 512, H + (h + 1) * 512)

            def sln(h):
                return slice(2 * H + h * 512, 2 * H + (h + 1) * 512)

            def dr_chain(p, sht8, w, colsl, start, stop, kps, soff=0):
                # fp8 DoubleRow accumulation over k-pairs; stationary pair j
                # of sht8 starts at slot soff+2j, moving pair at w[:, 2j:2j+2].
                kps = list(kps)
                for kp in kps:
                    nc.tensor.matmul(p, sht8[:, soff + 2 * kp:soff + 2 * kp + 2, :],
                                     w[:, 2 * kp:2 * kp + 2, colsl],
                                     start=(start and kp == kps[0]),
                                     stop=(stop and kp == kps[-1]), perf_mode=DR)

            def l0_h(h, tag):
                """L0 h-side chains for half h (need only old h0 in s_xh[2:10]).
                r/z psums left open (x-pair added later); gh complete.
                kp-outer: 3 consecutive matmuls share the stationary h-pair."""
                p_r = qq.tile([B, 512], F32, tag="ps", name=f"{tag}r")
                p_z = qq.tile([B, 512], F32, tag="ps", name=f"{tag}z")
                p_gh = qq.tile([B, 512], F32, tag="ps", name=f"{tag}gh")
                for kp in range(4):
                    for p, colsl, st in ((p_r, slr(h), False), (p_z, slz(h), False),
                                         (p_gh, sln(h), kp == 3)):
                        nc.tensor.matmul(p, s_xh[:, 2 + 2 * kp:4 + 2 * kp, :],
                                         s_wcat[:, 2 + 2 * kp:4 + 2 * kp, colsl],
                                         start=(kp == 0), stop=st, perf_mode=DR)
                return p_r, p_z, p_gh

            def l0_x(h, p_r, p_z, tag):
                """x-side contributions of half h (need s_xh slot 0)."""
                nc.tensor.matmul(p_r, s_xh[:, 0:2, :], s_wcat[:, 0:2, slr(h)],
                                 start=False, stop=True, perf_mode=DR)
                nc.tensor.matmul(p_z, s_xh[:, 0:2, :], s_wcat[:, 0:2, slz(h)],
                                 start=False, stop=True, perf_mode=DR)
                p_gi = qq.tile([B, 512], F32, tag="ps", name=f"{tag}gi")
                nc.tensor.matmul(p_gi, s_xh[:, 0:2, :], s_wcat[:, 0:2, sln(h)],
                                 start=True, stop=True, perf_mode=DR)
                return p_gi

            def l1_gh(h, tag):
                """gh1 for r,z,ghn of half h (only needs old h1t8); kp-outer
                so consecutive matmuls share the stationary pair."""
                p_r = qq.tile([B, 512], F32, tag="ps", name=f"{tag}r")
                p_z = qq.tile([B, 512], F32, tag="ps", name=f"{tag}z")
                p_gh = qq.tile([B, 512], F32, tag="ps", name=f"{tag}gh")
                for kp in range(4):
                    for p, colsl, st in ((p_r, slr(h), False), (p_z, slz(h), False),
                                         (p_gh, sln(h), kp == 3)):
                        nc.tensor.matmul(p, s_h1t8[:, 2 * kp:2 * kp + 2, :],
                                         s_whh1[:, 2 * kp:2 * kp + 2, colsl],
                                         start=(kp == 0), stop=st, perf_mode=DR)
                return p_r, p_z, p_gh

            def l1_gi_lo(h, p_r, p_z):
                """c0@Wih1 r/z k-pairs 0-1: needs only the FIRST L0 post."""
                dr_chain(p_r, s_xh, s_wih1, slr(h), False, False, (0, 1), soff=2)
                dr_chain(p_z, s_xh, s_wih1, slz(h), False, False, (0, 1), soff=2)

            def l1_gi_hi(h, p_r, p_z, tag):
                """c0@Wih1 rest (needs the SECOND L0 post)."""
                dr_chain(p_r, s_xh, s_wih1, slr(h), False, True, (2, 3), soff=2)
                dr_chain(p_z, s_xh, s_wih1, slz(h), False, True, (2, 3), soff=2)
                p_gi = qq.tile([B, 512], F32, tag="ps", name=f"{tag}gi")
                dr_chain(p_gi, s_xh, s_wih1, sln(h), True, True, range(4), soff=2)
                return p_gi

            def post_adds(h, p_r, p_z, p_gi, p_gh, s0t, bht, three_dve):
                """PSUM->SBUF stage. Only DVE (tensor_add, folds a constant
                for free) and ACT (plain copy) can read PSUM on real HW; the
                constants for ACT-copied tiles are folded later in transposed
                space (z: DVE on psum pT, gi: Pool on SBUF tn)."""
                c_r = tp.tile([B, 512], F16, tag="cp", bufs=8, name="c_r")
                c_z = tp.tile([B, 512], F16, tag="cp", bufs=8, name="c_z")
                c_gi = tp.tile([B, 512], F16, tag="cp", bufs=8, name="c_gi")
                c_gh = tp.tile([B, 512], F16, tag="cp", bufs=8, name="c_gh")
                nc.vector.tensor_add(c_r, p_r, s0t[:, slr(h)])
                nc.scalar.activation(c_gi, p_gi, AF.Copy)
                if three_dve:
                    nc.vector.tensor_add(c_z, p_z, s0t[:, slz(h)])
                else:
                    nc.scalar.activation(c_z, p_z, AF.Copy)
                    nc.gpsimd.tensor_add(c_z, c_z, s0t[:, slz(h)])
                nc.vector.tensor_add(c_gh, p_gh, bht[:, ts(h, 512)])
                return (c_r, c_z, c_gi, c_gh)

            def post_trans(cs, pT):
                for ki, src_ in enumerate(cs):
                    for q in range(4):
                        nc.tensor.transpose(pT[:, ki, q, :],
                                            src_[:, q * 128:(q + 1) * 128], s_id16)

            def post_math(h, pT, s0nT, s_htm, s_ht8, extra16, sbase):
                """Transposed gate math for half h of one layer. PSUM readers
                stay on DVE/ACT; the SBUF-only update chain runs on Pool."""
                rz = tp.tile([128, 2, 4, B], F16, tag="rz", bufs=4)
                nc.scalar.activation(rz, pT[:, 0:2], AF.Sigmoid, scale=INV)
                tn = tp.tile([128, 4, B], F32, tag="tn", bufs=4)
                nc.vector.tensor_mul(tn, rz[:, 0], pT[:, 3])
                nc.vector.tensor_add(tn, tn, pT[:, 2])
                nc.gpsimd.tensor_add(tn, tn, s0nT[:, h])
                nc.scalar.activation(tn, tn, AF.Tanh, scale=INV)
                m = s_htm[:, 4 * h:4 * h + 4, :]
                td = tp.tile([128, 4, B], F32, tag="td", bufs=4)
                nc.gpsimd.tensor_sub(td, m, tn)
                nc.gpsimd.tensor_mul(td, rz[:, 1], td)
                nc.gpsimd.tensor_add(m, tn, td)
                nc.scalar.activation(s_ht8[:, sbase + 4 * h:sbase + 4 * h + 4, :],
                                     m, AF.Copy)
                if extra16 is not None:
                    nc.gpsimd.tensor_copy(extra16[:, 4 * h:4 * h + 4, :], m)

            def fc_block(t_expr, pfcT, pxo):
                for k in range(KH):
                    nc.tensor.matmul(pfcT, s_fct[:, k, :], s_h1t16[:, k, :],
                                     start=(k == 0), stop=(k == KH - 1))
                # fp8 x for the gate chain first (critical path), then fp16 row
                nc.scalar.activation(s_xh[:, 0, :], pfcT, AF.Sigmoid, bias=s_fcb[:, :])
                nc.scalar.activation(s_xt, pfcT, AF.Sigmoid, bias=s_fcb[:, :])
                nc.tensor.transpose(pxo, s_xt, s_id128)
                s_xo = tp.tile([B, D], F32, tag="xo", bufs=3, name="s_xo")
                nc.scalar.activation(s_xo, pxo, AF.Copy)
                nc.sync.dma_start(out=d_out[:, ds(t_expr, 1), :],
                                  in_=s_xo.rearrange("b d -> b () d"))

            def step(t_expr, first):
                # L0 h-side matmuls (old h0) keep the PE busy while the
                # previous step's L1 gate math finishes on the vector engines.
                r0a, z0a, gh0a = l0_h(0, "a")
                r0b, z0b, gh0b = l0_h(1, "b")
                # psum slot-rotation control: allocate before the L1 chains so
                # each tile lands on an early-released bank (see FIFO notes).
                pfcT = qq.tile([D, B], F32, tag="ps", name="pfcT")
                pxo = qq.tile([B, D], F16, tag="ps", name="pxo")
                # previous step's fc -> x (fp8 slot) + output row
                if not first:
                    fc_block(t_expr - 1, pfcT, pxo)
                gi0a = l0_x(0, r0a, z0a, "a")
                gi0b = l0_x(1, r0b, z0b, "b")
                # L0 psum->sbuf stage for BOTH halves queues up front (DVE/ACT)
                cs0a = post_adds(0, r0a, z0a, gi0a, gh0a, s_s0, s_bh0b, True)
                cs0b = post_adds(1, r0b, z0b, gi0b, gh0b, s_s0, s_bh0b, False)
                pT0a = qq.tile([128, 4, 4, B], F16, tag="ps", name="pT0a")
                pT0b = qq.tile([128, 4, 4, B], F16, tag="ps", name="pT0b")
                # L1 gh matmuls (old h1) split around the L0 transposes so the
                # PE reaches each transpose block right as its adds finish.
                r1a, z1a, gh1a = l1_gh(0, "c")
                post_trans(cs0a, pT0a)
                post_math(0, pT0a, s_s0nT, s_h0tm, s_xh, None, 2)
                r1b, z1b, gh1b = l1_gh(1, "d")
                post_trans(cs0b, pT0b)
                post_math(1, pT0b, s_s0nT, s_h0tm, s_xh, None, 2)
                l1_gi_lo(0, r1a, z1a)
                l1_gi_lo(1, r1b, z1b)
                gi1a = l1_gi_hi(0, r1a, z1a, "c")
                gi1b = l1_gi_hi(1, r1b, z1b, "d")
                cs1a = post_adds(0, r1a, z1a, gi1a, gh1a, s_s1, s_bh1b, True)
                cs1b = post_adds(1, r1b, z1b, gi1b, gh1b, s_s1, s_bh1b, False)
                pT1a = qq.tile([128, 4, 4, B], F16, tag="ps", name="pT1a")
                pT1b = qq.tile([128, 4, 4, B], F16, tag="ps", name="pT1b")
                post_trans(cs1a, pT1a)
                post_math(0, pT1a, s_s1nT, s_h1tm, s_h1t8, s_h1t16, 0)
                post_trans(cs1b, pT1b)
                post_math(1, pT1b, s_s1nT, s_h1tm, s_h1t8, s_h1t16, 0)

            if static:
                for t in range(n_steps):
                    step(t, t == 0)
            else:
                step(0, True)
                while (n_steps - 1) % unroll != 0:
                    unroll -= 1
                with tc.For_i(1, n_steps, unroll,
                              hint_engines=(mybir.EngineType.PE,)) as iv:
                    for j in range(unroll):
                        step(iv + j, False)
            pfcT = qq.tile([D, B], F32, tag="ps", name="pfcT")
            pxo = qq.tile([B, D], F16, tag="ps", name="pxo")
            fc_block(n_steps - 1, pfcT, pxo)

            # ---------------- tail: trend/season + residual ----------------
            for b in range(B):
                for si in range(2):
                    base = TS_OFF + si * 128 * 512
                    par = (b * 2 + si) % 2
                    ps_o = qq.tile([128, D], F32, tag="ps")
                    for which in range(2):  # 0=trend 1=season
                        off = base + which * TS_LEN
                        src = d_embed[b:b + 1, off:off + 65536].rearrange(
                            "o (s f) -> (o s) f", f=512)
                        t_f = tp.tile([128, 512], F32, tag="tsf", bufs=4)
                        deng = (nc.sync, nc.scalar)[(par + which) % 2]
                        deng.dma_start(out=t_f, in_=src)
                        t_h = tp.tile([128, 512], F16, tag="tsh", bufs=4)
                        ceng = (nc.vector, nc.gpsimd)[(par + which) % 2]
                        ceng.tensor_copy(t_h, t_f)
                        # PE transpose [128,128] chunks (f on partitions)
                        p_tT = qq.tile([128, 4, 128], F16, tag="ps", name="p_tT")
                        for jj in range(4):
                            nc.tensor.transpose(p_tT[:, jj, :],
                                                t_h[:, jj * 128:(jj + 1) * 128],
                                                s_id128)
                        t_T = tp.tile([128, 4, 128], F16, tag="tst", bufs=4)
                        teng = (nc.vector, nc.scalar)[(par + which) % 2]
                        if teng is nc.scalar:
                            teng.activation(t_T, p_tT, AF.Copy)
                        else:
                            teng.tensor_copy(t_T, p_tT)
                        for jj in range(4):
                            nc.tensor.matmul(ps_o, t_T[:, jj, :], s_pred[:, jj, :],
                                             start=(which == 0 and jj == 0),
                                             stop=(which == 1 and jj == 3))
                    r_c = tp.tile([128, D], F32, tag="rc", bufs=4)
                    (nc.sync, nc.scalar)[par].dma_start(
                        out=r_c, in_=d_out[b, si * 128:(si + 1) * 128, :])
                    nc.vector.tensor_add(r_c, ps_o, r_c)
                    nc.gpsimd.tensor_add(r_c, r_c, s_pb2)
                    (nc.scalar, nc.sync)[par].dma_start(
                        out=d_out[b, si * 128:(si + 1) * 128, :], in_=r_c)

    nc.compile()
    return nc


def _prep_weights(W_ih0, W_hh0, b_ih0, b_hh0, W_ih1, W_hh1, b_ih1, b_hh1,
                  fc_W, fc_b, pred_W, pred_b):
    f16 = np.float16
    f8 = mybir.dt.np(F8)

    def karr(WT, dt, scale=1.0):  # [K, N] -> [128, K/128, N]
        K, N = WT.shape
        return np.ascontiguousarray(
            (WT * scale).reshape(K // 128, 128, N).transpose(1, 0, 2)).astype(dt)

    wx = (np.ascontiguousarray(W_ih0[:, H:H + D].T) * SCALE).astype(f8)
    wcat = np.concatenate(
        [wx[:, None, :], np.zeros((128, 1, G3), f8), karr(W_hh0.T, f8, SCALE)],
        axis=1)

    return dict(
        wcat0=np.ascontiguousarray(wcat),
        wg0=karr(W_ih0[:, :H].T, f8, SCALE),
        wih1=karr(W_ih1.T, f8, SCALE),
        whh1=karr(W_hh1.T, f8, SCALE),
        fct=karr(fc_W.T, f16),
        predt=np.ascontiguousarray(
            pred_W.T.reshape(4, 128, D).transpose(1, 0, 2)).astype(f16),
        s0b=(np.concatenate([(b_ih0 + b_hh0)[:2 * H], b_ih0[2 * H:]]) * SCALE).astype(f16),
        s1b=(np.concatenate([(b_ih1 + b_hh1)[:2 * H], b_ih1[2 * H:]]) * SCALE).astype(f16),
        bhh0n=(b_hh0[2 * H:] * SCALE).astype(f16),
        bhh1n=(b_hh1[2 * H:] * SCALE).astype(f16),
        fcb=np.ascontiguousarray(fc_b.reshape(D, 1)).astype(np.float32),
        id16=np.eye(B, dtype=np.float16),
        id128=np.eye(128, dtype=np.float16),
        pb2=(2.0 * pred_b).astype(np.float32),
    )


_NC_CACHE = {}


def kernel(embed, dynamics, W_ih0, W_hh0, b_ih0, b_hh0,
           W_ih1, W_hh1, b_ih1, b_hh1, fc_W, fc_b, pred_W, pred_b, seq_len,
           _n_steps=SEQ, _static=False, _trace=False):
    embed = np.asarray(embed, dtype=np.float32)
    dynamics = np.asarray(dynamics, dtype=np.float32)
    wd = _prep_weights(np.asarray(W_ih0, np.float32), np.asarray(W_hh0, np.float32),
                       np.asarray(b_ih0, np.float32), np.asarray(b_hh0, np.float32),
                       np.asarray(W_ih1, np.float32), np.asarray(W_hh1, np.float32),
                       np.asarray(b_ih1, np.float32), np.asarray(b_hh1, np.float32),
                       np.asarray(fc_W, np.float32), np.asarray(fc_b, np.float32),
                       np.asarray(pred_W, np.float32), np.asarray(pred_b, np.float32))

    key = (_n_steps, _static)
    if key not in _NC_CACHE:
        _NC_CACHE[key] = build_nc(n_steps=_n_steps, static=_static)
    nc = _NC_CACHE[key]

    in_maps = []
    for c in range(NCORES):
        m = dict(wd)
        m["embed"] = np.ascontiguousarray(embed[c * B:(c + 1) * B])
        m["x0"] = np.ascontiguousarray(dynamics[c * B:(c + 1) * B, 0, :])
        in_maps.append(m)

    res = run_bass_kernel_spmd(nc, in_maps, list(range(NCORES)), trace=False)
    out = np.concatenate([res.results[c]["out"] for c in range(NCORES)], axis=0)
    if _trace:
        kernel.last_exec_time_ns = _bench_exec(nc, in_maps)
    return out


def _bench_exec(nc, in_maps, n_reps=5, k_lo=4, k_hi=20):
    """Steady-state per-execution hardware time of the sharded NEFF.

    The NTFF profiling hook is unavailable under this axon client and a
    single dispatch carries ~40-80ms of client<->terminal RPC latency, so
    a single timed call measures mostly RPC overhead. Instead dispatch
    chains of k_lo and k_hi executions asynchronously (device executions
    queue back-to-back), block once, and report the marginal time per
    execution (T(k_hi) - T(k_lo)) / (k_hi - k_lo), min over n_reps."""
    import time

    import jax
    from jax.sharding import Mesh, NamedSharding, PartitionSpec
    from jax.experimental.shard_map import shard_map

    from concourse import bass2jax, mybir as _mb

    bass2jax.install_neuronx_cc_hook()
    n_cores = len(in_maps)
    partition_name = (nc.partition_id_tensor.name if nc.partition_id_tensor else None)
    in_names, out_names, out_avals, zero_outs = [], [], [], []
    for alloc in nc.m.functions[0].allocations:
        if not isinstance(alloc, _mb.MemoryLocationSet):
            continue
        name = alloc.memorylocations[0].name
        if alloc.kind == "ExternalInput":
            if name != partition_name:
                in_names.append(name)
        elif alloc.kind == "ExternalOutput":
            out_names.append(name)
            shape = tuple(alloc.tensor_shape)
            dtype = _mb.dt.np(alloc.dtype)
            out_avals.append(jax.core.ShapedArray(shape, dtype))
            zero_outs.append(np.zeros(shape, dtype))
    n_params = len(in_names)
    all_names = list(in_names) + out_names
    if partition_name is not None:
        all_names.append(partition_name)

    def _body(*args):
        operands = list(args)
        if partition_name is not None:
            operands.append(bass2jax.partition_id_tensor())
        return tuple(bass2jax._bass_exec_p.bind(
            *operands,
            out_avals=tuple(out_avals),
            in_names=tuple(all_names),
            out_names=tuple(out_names),
            lowering_input_output_aliases=(),
            sim_require_finite=False,
            sim_require_nnan=False,
            nc=nc,
        ))

    devices = jax.devices()[:n_cores]
    mesh = Mesh(np.asarray(devices), ("core",))
    spec = PartitionSpec("core")
    fn = jax.jit(shard_map(
        _body, mesh=mesh,
        in_specs=(spec,) * (n_params + len(out_names)),
        out_specs=(spec,) * len(out_names), check_rep=False))
    sh = NamedSharding(mesh, spec)
    dev_in = [jax.device_put(
        np.concatenate([np.asarray(in_maps[c][nm]) for c in range(n_cores)], axis=0), sh)
        for nm in in_names]
    dev_zo = [jax.device_put(np.concatenate([z] * n_cores, axis=0), sh) for z in zero_outs]
    r = fn(*dev_in, *dev_zo)
    jax.block_until_ready(r)

    def chain(k):
        best = float("inf")
        for _ in range(n_reps):
            t0 = time.perf_counter()
            rs = [fn(*dev_in, *dev_zo) for _ in range(k)]
            jax.block_until_ready(rs)
            best = min(best, time.perf_counter() - t0)
        return best

    t_lo = chain(k_lo)
    t_hi = chain(k_hi)
    return int((t_hi - t_lo) / (k_hi - k_lo) * 1e9)
